# revision 1
# baseline (speedup 1.0000x reference)
"""Self-contained kernel for nn_Attention_17300128268584.

Strategy (per sharding hint): data-parallel over batch B=16 across the
8 NeuronCores (B=2 per core); weights/table replicated. The full
conv->GN->GELU projections, relative-position-biased attention, and the
1x1 output projection run on-device per shard; results are gathered and
concatenated on the host.

Hardcoded problem shape: x (16,128,32,32) f32, 4 heads x 32 dim,
N = 32*32 = 1024 tokens.
"""

import numpy as np

EPS = 1e-6
DIM_HEAD = 32
N_CORES = 8


def _shard_fn(x, wq, gq, bq, wk, gk, bk, wv, gv, bv, table, rel_index, out_w, out_b):
    # Runs on ONE core with a batch shard x: (2, 128, 32, 32).
    import jax
    import jax.numpy as jnp

    B, C, ih, iw = x.shape
    H = C // DIM_HEAD
    N = ih * iw

    def conv3x3(t, w):
        return jax.lax.conv_general_dilated(
            t, w, window_strides=(1, 1), padding=((1, 1), (1, 1)),
            dimension_numbers=("NCHW", "OIHW", "NCHW"))

    def gn1(t, gamma, beta):
        m = jnp.mean(t, axis=(1, 2, 3), keepdims=True)
        v = jnp.var(t, axis=(1, 2, 3), keepdims=True)
        tn = (t - m) * jax.lax.rsqrt(v + EPS)
        return tn * gamma[None, :, None, None] + beta[None, :, None, None]

    def proj(t, w, gamma, beta):
        return jax.nn.gelu(gn1(conv3x3(t, w), gamma, beta), approximate=False)

    def to_heads(t):
        return (t.reshape(B, C, N).transpose(0, 2, 1)
                 .reshape(B, N, H, DIM_HEAD).transpose(0, 2, 1, 3))

    q = to_heads(proj(x, wq, gq, bq))
    k = to_heads(proj(x, wk, gk, bk))
    v = to_heads(proj(x, wv, gv, bv))

    dots = jnp.einsum("bhnd,bhmd->bhnm", q, k)
    bias = table[rel_index].reshape(N, N, H).transpose(2, 0, 1)[None]
    attn = jax.nn.softmax(dots + bias, axis=-1)
    out = jnp.einsum("bhnm,bhmd->bhnd", attn, v)
    out = (out.transpose(0, 2, 1, 3).reshape(B, N, C)
              .transpose(0, 2, 1).reshape(B, C, ih, iw))
    out = jax.lax.conv_general_dilated(
        out, out_w, window_strides=(1, 1), padding=((0, 0), (0, 0)),
        dimension_numbers=("NCHW", "OIHW", "NCHW")) + out_b[None, :, None, None]
    return out


def _run_device(inputs):
    import jax

    jax.config.update("jax_default_matmul_precision", "highest")
    devs = [d for d in jax.devices() if d.platform != "cpu"]
    if len(devs) < N_CORES:
        raise RuntimeError(f"need {N_CORES} accelerator cores, have {len(devs)}")
    devs = devs[:N_CORES]

    x = np.asarray(inputs["x"], np.float32)
    B = x.shape[0]
    assert B % N_CORES == 0
    xs = x.reshape(N_CORES, B // N_CORES, *x.shape[1:])

    names = ["wq", "gq", "bq", "wk", "gk", "bk", "wv", "gv", "bv",
             "table", "rel_index", "out_w", "out_b"]
    reps = [np.asarray(inputs[n]) for n in names]

    pm = jax.pmap(
        _shard_fn,
        in_axes=(0,) + (None,) * len(names),
        devices=devs,
    )
    out = pm(xs, *reps)
    out = np.asarray(out, np.float32)
    return out.reshape(B, *out.shape[2:])


def _run_host(inputs):
    import jax

    with jax.default_device(jax.devices("cpu")[0]):
        out = jax.jit(_shard_fn, backend="cpu")(
            *[np.asarray(inputs[n]) for n in
              ["x", "wq", "gq", "bq", "wk", "gk", "bk", "wv", "gv", "bv",
               "table", "rel_index", "out_w", "out_b"]])
    return np.asarray(out, np.float32)


def kernel(**inputs) -> np.ndarray:
    try:
        return _run_device(inputs)
    except Exception:
        return _run_host(inputs)



# revision 3
# speedup vs baseline: 1.3895x; 1.3895x over previous
"""Self-contained kernel for nn_Attention_17300128268584.

Strategy: data-parallel over batch B=16 across 8 NeuronCores (2 samples per
core). A single fused Bass/Tile kernel per core runs the whole module:
3x (conv3x3 -> GroupNorm(1 group) -> GELU) projections, relative-position-
biased 4-head attention, and the 1x1 output conv. It is compiled once per
process (bass_jit -> bass_exec custom call -> NEFF, disk-cached) and wrapped
in a jax shard_map over the 8 axon cores. Weights and the precomputed
exp(bias) table are pushed to device HBM once and kept resident; warm calls
only ship x (fp16, 4MB) up and the output (fp16, 4MB) down.

Attention layout trick: S^T = K Q^T is computed with keys on PSUM partitions
(lhsT=k-chunk, rhs=q), so the exp tiles P=(keys, queries) feed the PV matmul
directly with no transposes of P. V chunks are PE-transposed once per sample
and augmented with a ones-column so the PV matmul accumulates the softmax
denominators alongside the weighted values.
"""

from contextlib import ExitStack

import numpy as np

N_CORES = 8
B = 16
C = 128
N = 1024
H_IMG = 32
HEADS = 4
NKC = N // 128
EPS = 1e-6

_CACHE = {}


# ---------------------------------------------------------------- bass kernel

def _attn_body(ctx, tc, out_ap, x_ap, w_ap, gnp_ap, ebT_ap, outw_ap,
               outb_ap, id_ap):
    import concourse.mybir as mybir

    FP16 = mybir.dt.float16
    BF16 = mybir.dt.bfloat16
    F32 = mybir.dt.float32
    AF = mybir.ActivationFunctionType
    AX = mybir.AxisListType

    nc = tc.nc
    BS = x_ap.shape[0]

    consts = ctx.enter_context(tc.tile_pool(name="consts", bufs=1))
    persist = ctx.enter_context(tc.tile_pool(name="persist", bufs=1))
    work = ctx.enter_context(tc.tile_pool(name="work", bufs=2))
    small = ctx.enter_context(tc.tile_pool(name="small", bufs=2))
    ebp = ctx.enter_context(tc.tile_pool(name="ebp", bufs=10))
    pp = ctx.enter_context(tc.tile_pool(name="pp", bufs=4))
    psum = ctx.enter_context(tc.tile_pool(name="psum", bufs=3, space="PSUM"))
    psum_o = ctx.enter_context(tc.tile_pool(name="psum_o", bufs=2, space="PSUM"))
    psum_b = ctx.enter_context(tc.tile_pool(name="psum_b", bufs=1, space="PSUM"))

    # constants to SBUF
    w_sb = consts.tile([C, 27, C], FP16, name="w", tag="w")
    nc.sync.dma_start(w_sb[:], w_ap[:])
    gnp_sb = consts.tile([C, 6], F32, name="gnp", tag="gnp")
    nc.sync.dma_start(gnp_sb[:], gnp_ap[:])
    outw_sb = consts.tile([C, C], BF16, name="outw", tag="outw")
    nc.sync.dma_start(outw_sb[:], outw_ap[:])
    outb_sb = consts.tile([C, 1], F32, name="outb", tag="outb")
    nc.sync.dma_start(outb_sb[:], outb_ap[:])
    id_sb = consts.tile([C, 32], BF16, name="ident", tag="ident")
    nc.sync.dma_start(id_sb[:], id_ap[:])
    ones_sb = consts.tile([C, C], F32, name="ones", tag="ones")
    nc.vector.memset(ones_sb[:], 1.0)
    junk_sb = consts.tile([C, 512], F32, name="junk", tag="junk")

    # persistent per-sample tiles
    q_sb = [persist.tile([C, N], FP16, name=f"q{s}", tag=f"q{s}") for s in range(BS)]
    k_sb = [persist.tile([C, N], FP16, name=f"k{s}", tag=f"k{s}") for s in range(BS)]
    v_sb = [persist.tile([C, N], BF16, name=f"v{s}", tag=f"v{s}") for s in range(BS)]
    y_sb = [persist.tile([C, N], BF16, name=f"y{s}", tag=f"y{s}") for s in range(BS)]
    va_sb = [[persist.tile([C, NKC, 33], BF16, name=f"va{s}_{h}", tag=f"va{s}_{h}")
              for h in range(HEADS)] for s in range(BS)]

    # phase 1: conv3x3 + GN(1 group) + GELU for q, k, v
    for s in range(BS):
        xp = work.tile([C, 34, 34], FP16, name="xpad", tag="xpad")
        nc.vector.memset(xp[:], 0.0)
        nc.sync.dma_start(
            xp[:, 1:33, 1:33],
            x_ap[s].rearrange("c (h w) -> c h w", h=H_IMG),
        )
        for p in range(3):
            halves = []
            for hf in range(2):
                ps = psum.tile([C, 512], F32, name="mm", tag="mm")
                for t in range(9):
                    ky, kx = divmod(t, 3)
                    rhs = xp[:, 16 * hf + ky: 16 * hf + ky + 16, kx: kx + 32]
                    nc.tensor.matmul(
                        ps[:], lhsT=w_sb[:, 9 * p + t, :], rhs=rhs,
                        start=(t == 0), stop=(t == 8),
                    )
                halves.append(ps)
            part = small.tile([C, 4], F32, name="part", tag="part")
            for hf in range(2):
                nc.vector.reduce_sum(part[:, 2 * hf: 2 * hf + 1], halves[hf][:],
                                     axis=AX.X)
                nc.scalar.activation(junk_sb[:], halves[hf][:], AF.Square,
                                     accum_out=part[:, 2 * hf + 1: 2 * hf + 2])
            s_all = small.tile([C, 2], F32, name="sall", tag="sall")
            nc.vector.tensor_add(s_all[:], part[:, 0:2], part[:, 2:4])
            pb = psum_b.tile([C, 2], F32, name="bc", tag="bc")
            nc.tensor.matmul(pb[:], lhsT=ones_sb[:], rhs=s_all[:],
                             start=True, stop=True)
            stat = small.tile([C, 2], F32, name="stat", tag="stat")
            nc.vector.tensor_scalar_mul(stat[:], pb[:], 1.0 / (C * N))
            var = small.tile([C, 1], F32, name="var", tag="var")
            nc.vector.tensor_mul(var[:], stat[:, 0:1], stat[:, 0:1])
            nc.vector.tensor_sub(var[:], stat[:, 1:2], var[:])
            nc.vector.tensor_scalar_add(var[:], var[:], EPS)
            std = small.tile([C, 1], F32, name="std", tag="std")
            nc.scalar.sqrt(std[:], var[:])
            rstd = small.tile([C, 1], F32, name="rstd", tag="rstd")
            nc.vector.reciprocal(rstd[:], std[:])
            scl = small.tile([C, 1], F32, name="scl", tag="scl")
            nc.vector.tensor_mul(scl[:], gnp_sb[:, 2 * p: 2 * p + 1], rstd[:])
            bia = small.tile([C, 1], F32, name="bia", tag="bia")
            nc.vector.tensor_mul(bia[:], stat[:, 0:1], scl[:])
            nc.vector.tensor_sub(bia[:], gnp_sb[:, 2 * p + 1: 2 * p + 2], bia[:])
            dst = (q_sb, k_sb, v_sb)[p][s]
            for hf in range(2):
                nc.scalar.activation(dst[:, 512 * hf: 512 * (hf + 1)],
                                     halves[hf][:], AF.Gelu,
                                     bias=bia[:], scale=scl[:])

    # phase 2: transpose v into augmented [keys, d | 1] chunks
    for s in range(BS):
        for h in range(HEADS):
            va = va_sb[s][h]
            nc.vector.memset(va[:, :, 32:33], 1.0)
            for kc in range(NKC):
                pvt = psum.tile([C, 32], BF16, name="vt", tag="vt", bufs=2)
                nc.tensor.transpose(
                    pvt[:],
                    v_sb[s][32 * h: 32 * h + 32, 128 * kc: 128 * (kc + 1)],
                    id_sb[32 * h: 32 * h + 32, :],
                    tile_position=(32 * h, 0),
                )
                nc.vector.tensor_copy(va[:, kc, 0:32], pvt[:])

    # phase 3: attention per head
    for h in range(HEADS):
        ebts = []
        for kc in range(NKC):
            ebt = ebp.tile([C, N], BF16, name="eb", tag="eb")
            nc.sync.dma_start(ebt[:], ebT_ap[h, 128 * kc: 128 * (kc + 1), :])
            ebts.append(ebt)
        for s in range(BS):
            for qh in range(2):
                po = psum_o.tile([33, 512], F32, name="po", tag="po")
                for kc in range(NKC):
                    pS = psum.tile([C, 512], F32, name="mm", tag="mm")
                    nc.tensor.matmul(
                        pS[:],
                        lhsT=k_sb[s][32 * h: 32 * h + 32, 128 * kc: 128 * (kc + 1)],
                        rhs=q_sb[s][32 * h: 32 * h + 32, 512 * qh: 512 * (qh + 1)],
                        start=True, stop=True,
                        tile_position=(32 * h, 0),
                    )
                    pt = pp.tile([C, 512], BF16, name="p", tag="p")
                    nc.scalar.activation(pt[:], pS[:], AF.Exp)
                    nc.vector.tensor_mul(pt[:], pt[:],
                                         ebts[kc][:, 512 * qh: 512 * (qh + 1)])
                    nc.tensor.matmul(po[:], lhsT=va_sb[s][h][:, kc, :], rhs=pt[:],
                                     start=(kc == 0), stop=(kc == NKC - 1))
                inv = small.tile([1, 512], F32, name="inv", tag="inv")
                nc.vector.reciprocal(inv[:], po[32:33, :])
                pbc = psum.tile([C, 512], F32, name="mm", tag="mm")
                nc.tensor.matmul(pbc[:, 0:512], lhsT=ones_sb[0:1, :], rhs=inv[:],
                                 start=True, stop=True)
                nc.vector.tensor_mul(
                    y_sb[s][32 * h: 32 * h + 32, 512 * qh: 512 * (qh + 1)],
                    po[0:32, :], pbc[0:32, :])

    # phase 4: 1x1 output conv + bias
    for s in range(BS):
        o_sb = work.tile([C, N], FP16, name="osb", tag="osb")
        for qh in range(2):
            pf = psum.tile([C, 512], F32, name="mm", tag="mm")
            nc.tensor.matmul(pf[:], lhsT=outw_sb[:],
                             rhs=y_sb[s][:, 512 * qh: 512 * (qh + 1)],
                             start=True, stop=True)
            nc.scalar.activation(o_sb[:, 512 * qh: 512 * (qh + 1)], pf[:],
                                 AF.Identity, bias=outb_sb[:, 0:1], scale=1.0)
        nc.sync.dma_start(out_ap[s], o_sb[:])


# ------------------------------------------------------------- host-side prep

def _prep_constants(inputs):
    import ml_dtypes
    W = np.stack([np.asarray(inputs["wq"], np.float32),
                  np.asarray(inputs["wk"], np.float32),
                  np.asarray(inputs["wv"], np.float32)])  # (3,O,I,3,3)
    wqkv = np.ascontiguousarray(
        W.transpose(2, 0, 3, 4, 1).reshape(C, 27, C)).astype(np.float16)
    gnp = np.stack([np.asarray(inputs[k], np.float32) for k in
                    ("gq", "bq", "gk", "bk", "gv", "bv")], axis=1)  # (128, 6)
    table = np.asarray(inputs["table"], np.float32)
    rel = np.asarray(inputs["rel_index"])
    bias_full = table[rel]  # (N*N, H)
    ebT = np.exp(bias_full.reshape(N, N, HEADS)).transpose(2, 1, 0)  # [h,key,qry]
    ebT = np.ascontiguousarray(ebT).astype(ml_dtypes.bfloat16)
    outw = np.ascontiguousarray(
        np.asarray(inputs["out_w"], np.float32)[:, :, 0, 0].T
    ).astype(ml_dtypes.bfloat16)
    outb = np.asarray(inputs["out_b"], np.float32).reshape(C, 1)
    ident = np.concatenate([np.eye(32, dtype=np.float32)] * 4,
                           axis=0).astype(ml_dtypes.bfloat16)
    return dict(wqkv=wqkv, gnp=gnp, ebT=ebT, outw=outw, outb=outb, ident=ident)


_CONST_KEYS = ("wq", "wk", "wv", "gq", "bq", "gk", "bk", "gv", "bv",
               "table", "rel_index", "out_w", "out_b")


def _consts_match(inputs, cached_inputs):
    for k in _CONST_KEYS:
        a, b = np.asarray(inputs[k]), cached_inputs[k]
        if a.shape != b.shape or a.dtype != b.dtype or not np.array_equal(a, b):
            return False
    return True


def _build_state(inputs):
    import jax
    import concourse.mybir as mybir
    import concourse.tile as tile
    from concourse.bass2jax import bass_jit, bass_shard_map
    from jax.sharding import Mesh, PartitionSpec as P, NamedSharding

    devs = jax.devices()[:N_CORES]
    if len(devs) < N_CORES:
        raise RuntimeError(f"need {N_CORES} cores, have {len(devs)}")
    mesh = Mesh(np.asarray(devs), ("core",))

    @bass_jit
    def attn_fn(nc, x, wqkv, gnp, ebT, outw, outb, ident):
        out = nc.dram_tensor("out", [x.shape[0], C, N], mybir.dt.float16,
                             kind="ExternalOutput")
        with tile.TileContext(nc) as tc:
            with ExitStack() as ctx:
                _attn_body(ctx, tc, out.ap(), x, wqkv, gnp, ebT, outw,
                           outb, ident)
        return out

    f = bass_shard_map(
        attn_fn,
        mesh=mesh,
        in_specs=(P("core"), P(), P(), P(), P(), P(), P()),
        out_specs=P("core"),
    )

    consts = _prep_constants(inputs)
    rep = NamedSharding(mesh, P())
    const_dev = [jax.device_put(consts[k], rep) for k in
                 ("wqkv", "gnp", "ebT", "outw", "outb", "ident")]

    state = {
        "f": f,
        "mesh": mesh,
        "xsh": NamedSharding(mesh, P("core")),
        "const_dev": const_dev,
        "inputs": {k: np.copy(np.asarray(inputs[k])) for k in _CONST_KEYS},
    }
    return state


def _run_bass(inputs):
    import jax

    x16 = np.asarray(inputs["x"], np.float32).reshape(B, C, N).astype(np.float16)

    state = _CACHE.get("state")
    if state is not None and not _consts_match(inputs, state["inputs"]):
        state = None
    if state is None:
        state = _build_state(inputs)
        _CACHE["state"] = state

    xd = jax.device_put(x16, state["xsh"])
    out = state["f"](xd, *state["const_dev"])
    o = np.asarray(out)  # (16, 128, 1024) fp16
    o32 = o.astype(np.float32).reshape(B, C, H_IMG, H_IMG)
    if not np.isfinite(o32).all():
        raise FloatingPointError("non-finite output from bass kernel")
    return o32


# ------------------------------------------------------- fallback (jax pmap)

def _shard_fn(x, wq, gq, bq, wk, gk, bk, wv, gv, bv, table, rel_index, out_w,
              out_b):
    import jax
    import jax.numpy as jnp

    Bs, Cc, ih, iw = x.shape
    H = Cc // 32
    Nn = ih * iw

    def conv3x3(t, w):
        return jax.lax.conv_general_dilated(
            t, w, window_strides=(1, 1), padding=((1, 1), (1, 1)),
            dimension_numbers=("NCHW", "OIHW", "NCHW"))

    def gn1(t, gamma, beta):
        m = jnp.mean(t, axis=(1, 2, 3), keepdims=True)
        v = jnp.var(t, axis=(1, 2, 3), keepdims=True)
        tn = (t - m) * jax.lax.rsqrt(v + EPS)
        return tn * gamma[None, :, None, None] + beta[None, :, None, None]

    def proj(t, w, gamma, beta):
        return jax.nn.gelu(gn1(conv3x3(t, w), gamma, beta), approximate=False)

    def to_heads(t):
        return (t.reshape(Bs, Cc, Nn).transpose(0, 2, 1)
                .reshape(Bs, Nn, H, 32).transpose(0, 2, 1, 3))

    q = to_heads(proj(x, wq, gq, bq))
    k = to_heads(proj(x, wk, gk, bk))
    v = to_heads(proj(x, wv, gv, bv))

    dots = jnp.einsum("bhnd,bhmd->bhnm", q, k)
    bias = table[rel_index].reshape(Nn, Nn, H).transpose(2, 0, 1)[None]
    attn = jax.nn.softmax(dots + bias, axis=-1)
    out = jnp.einsum("bhnm,bhmd->bhnd", attn, v)
    out = (out.transpose(0, 2, 1, 3).reshape(Bs, Nn, Cc)
           .transpose(0, 2, 1).reshape(Bs, Cc, ih, iw))
    out = jax.lax.conv_general_dilated(
        out, out_w, window_strides=(1, 1), padding=((0, 0), (0, 0)),
        dimension_numbers=("NCHW", "OIHW", "NCHW")) + out_b[None, :, None, None]
    return out


def _run_fallback(inputs):
    import jax

    names = ["wq", "gq", "bq", "wk", "gk", "bk", "wv", "gv", "bv",
             "table", "rel_index", "out_w", "out_b"]
    try:
        devs = [d for d in jax.devices() if d.platform != "cpu"][:N_CORES]
        x = np.asarray(inputs["x"], np.float32)
        xs = x.reshape(N_CORES, B // N_CORES, *x.shape[1:])
        pm = jax.pmap(_shard_fn, in_axes=(0,) + (None,) * len(names),
                      devices=devs)
        out = np.asarray(pm(xs, *[np.asarray(inputs[n]) for n in names]),
                         np.float32)
        return out.reshape(B, *out.shape[2:])
    except Exception:
        with jax.default_device(jax.devices("cpu")[0]):
            out = jax.jit(_shard_fn, backend="cpu")(
                np.asarray(inputs["x"]),
                *[np.asarray(inputs[n]) for n in names])
        return np.asarray(out, np.float32)


def kernel(**inputs) -> np.ndarray:
    try:
        return _run_bass(inputs)
    except Exception:
        return _run_fallback(inputs)
